# revision 6
# baseline (speedup 1.0000x reference)
"""Trainium2 Bass kernel for BondUpdate GNN message passing.

Computes, for each bond e:
    x = concat(sites[idx1[e]], sites[idx2[e]], bonds[e], states[g2b[e]])  # [896]
    out[e] = relu(relu(relu(x @ W1 + b1) @ W2 + b2) @ W3 + b3)           # [256]

Strategy (v3): the 20000 sites are referenced ~400k times via idx1/idx2, and
512 graph states ~200k times, so the W1 blocks that multiply site/state
features are precomputed per core into DRAM projection tables:
    A1 = sites @ W1[0:256]      (only the local idx1 range)
    A2 = sites @ W1[256:512]    (deduped idx2 sites for this core)
    Ast = states @ W1[768:896]
Per bond, layer 1 then reduces to a 256-wide matmul on the bond features plus
a gather+sum of three projection rows, transposed into feature-major via the
PE. Bonds are sharded across 8 cores by idx1 range (so A1 is small), and
WITHIN each core sorted by idx2, so each tile's A2 rows form a narrow window
of the (rank-ordered) dedup table. The A2 table is then built as overlapping
2048-rank chunks (span 3584 rows) in separate DRAM tensors, and chunk
projection work is interleaved INTO the main loop just-in-time: the tensor
engine never sits idle waiting for the whole A2 table (the v2 kernel lost
~75us to precompute-phase stalls). Chunk choice per tile is shared across
cores (SPMD single program): the +-512-rank margin absorbs cross-core rank
fluctuations.

Activations stay transposed in SBUF (features on partitions, bonds on free
dim) so the three matmul layers chain without intermediate transposes.
Matmul operands are bf16 (PSUM accumulation fp32, biases+relu applied in
fp32, final output stage f32r so values are not re-rounded).
"""
import sys

if "/opt/trn_rl_repo" not in sys.path:
    sys.path.insert(0, "/opt/trn_rl_repo")

import ml_dtypes
import numpy as np

import concourse.bass as bass
import concourse.mybir as mybir
import concourse.tile as tile
from concourse.bass_utils import run_bass_kernel_spmd
from concourse.masks import make_identity
from concourse.vector_clock import ScopedClock

F32 = mybir.dt.float32
F32R = mybir.dt.float32r
BF16 = mybir.dt.bfloat16
I32 = mybir.dt.int32

P = 128            # partitions
T = 512            # bonds per tile
SUB = T // P       # 128-bond subtiles per tile

N_SITES = 20000
N_GRAPHS = 512
SITE_LEN = 256
BOND_LEN = 256
STATE_LEN = 128
H1 = 1024
H2 = 1024
OUT_DIM = 256

KCB, MC1 = BOND_LEN // P, H1 // P  # 2, 8   (bond block of W1)
KC2, MC2 = H1 // P, H2 // P        # 8, 8
KC3, MC3 = H2 // P, OUT_DIM // P   # 8, 2

N_CORES = 8
N_BONDS = 200000
E_SHARD = N_BONDS // N_CORES       # 25000 bonds per core
TILES_PER_CORE = 49                # 49*512 = 25088 >= 25000
GROUP = 512                        # table rows per projection group
CHUNK_STEP = 2048                  # rank stride between A2 chunks
CHUNK_MARGIN = 512                 # low-side rank margin per chunk
CHUNK_SPAN = 3584                  # rows covered by one chunk tensor
SCHED_CUSHION = 3                  # emit chunk groups >= this many tiles early

EVSEM_WAIT_CAP = 2  # InstEventSemaphore holds 2 waits; every other inst 1


def _legalize_waits(nc: bass.Bass):
    """Spill sync waits beyond the per-instruction capacity onto standalone
    InstEventSemaphore instructions inserted just before the offender.
    Walrus here rejects instructions with more waits than the ISA slots."""
    n_spilled = 0
    for f in nc.m.functions:
        for bb in f.blocks:
            il = bb.instructions
            i = 0
            while i < len(il):
                inst = il[i]
                si = inst.sync_info
                waits = list(si.on_wait) if si and si.on_wait else []
                cap = (
                    EVSEM_WAIT_CAP
                    if isinstance(inst, mybir.InstEventSemaphore)
                    else 1
                )
                if len(waits) > cap:
                    keep = waits[-cap:]
                    spill = waits[:-cap]
                    si.on_wait = keep
                    evs = []
                    for j in range(0, len(spill), EVSEM_WAIT_CAP):
                        ev = mybir.InstEventSemaphore(
                            name=nc.get_next_instruction_name(),
                            ins=[],
                            outs=[],
                            sync_info=mybir.SyncInfo(
                                on_wait=spill[j:j + EVSEM_WAIT_CAP],
                                on_update=[],
                            ),
                        )
                        ev.engine = inst.engine
                        nc.register_instruction(ev)
                        evs.append(ev)
                    il[i:i] = evs
                    i += len(evs)
                    n_spilled += len(spill)
                i += 1
    return n_spilled


class SplitDrainTileContext(tile.TileContext):
    """TileContext whose kernel-tail drain also respects the wait cap."""

    def _drain_and_barrier(self, tick_clock, wait_clock):
        nc = self.nc
        drain_inst = nc.sync.drain()
        wait_clock.add_sem_waits(
            drain_inst.ins, ScopedClock({None: tick_clock.global_clock})
        )
        si = drain_inst.ins.sync_info
        waits = list(si.on_wait or [])
        if len(waits) > 1:
            si.on_wait = []
            id2sem = {s.num: s for s in self.sems.allocated().values()}
            for w in waits:
                assert w.wait_mode == "sem-ge-imm", w
                nc.sync.wait_ge(id2sem[w.id], w.wait_value)
        nc.all_engine_barrier()
        assert self.sems is not None
        popped = nc._tile_sem_poison_stack.pop()
        assert popped is self._sem_poison
        nc.clear_and_free_semaphores(list(self.sems.allocated().values()))
        nc.all_engine_barrier()


def build_bass(n_tiles: int, nloc: int, nch2g: int,
               chunk_meta: tuple, chunk_of: tuple,
               sched_pre: tuple, sched_main: tuple) -> bass.Bass:
    """Per-core Bass program.

    chunk_meta: tuple of (base_row, n_rows) per A2 chunk tensor.
    chunk_of:   per tile, which chunk its idx2 gathers read.
    sched_pre:  A2 group ids emitted before the main loop.
    sched_main: per tile, tuple of A2 group ids emitted after that tile.
    """
    nc = bass.Bass("TRN2", target_bir_lowering=False, debug=False, num_devices=1)
    E = n_tiles * T
    LSITE = nloc * P
    LS2 = nch2g * GROUP
    NCH = len(chunk_meta)

    # --- external inputs
    sitesT2 = nc.dram_tensor("sitesT2", [SITE_LEN, LS2], BF16, kind="ExternalInput")
    sitesTloc = nc.dram_tensor("sitesTloc", [SITE_LEN, LSITE], BF16, kind="ExternalInput")
    statesT = nc.dram_tensor("statesT", [STATE_LEN, N_GRAPHS], BF16, kind="ExternalInput")
    bondsT = nc.dram_tensor("bondsT", [BOND_LEN, E], BF16, kind="ExternalInput")
    # indices pre-wrapped on host to [P, n_tiles*SUB]: idx[p, t*SUB+j] = raw[t*T + j*P + p]
    idx1 = nc.dram_tensor("idx1", [P, n_tiles * SUB], I32, kind="ExternalInput")
    idx2 = nc.dram_tensor("idx2", [P, n_tiles * SUB], I32, kind="ExternalInput")
    g2b = nc.dram_tensor("g2b", [P, n_tiles * SUB], I32, kind="ExternalInput")
    # W1 site block rows 0:512 as [p, k, h] = W1[k*128+p, h], k=0..3 (bf16)
    w1s = nc.dram_tensor("w1s", [P, 4, H1], BF16, kind="ExternalInput")
    # W1 state block rows 768:896: [p, h] = W1[768+p, h]
    w1st = nc.dram_tensor("w1st", [P, H1], BF16, kind="ExternalInput")
    # W1 bond block rows 512:768 chunked: w1c[p, (k*MC1+m)*P+j] = W1[512+k*P+p, m*P+j]
    w1c = nc.dram_tensor("w1c", [P, KCB * MC1 * P], BF16, kind="ExternalInput")
    w2c = nc.dram_tensor("w2c", [P, KC2 * MC2 * P], BF16, kind="ExternalInput")
    w3c = nc.dram_tensor("w3c", [P, KC3 * MC3 * P], BF16, kind="ExternalInput")
    # biases pre-wrapped: bXc[p, m] = bX[m*P+p]
    b1c = nc.dram_tensor("b1c", [P, MC1], F32, kind="ExternalInput")
    b2c = nc.dram_tensor("b2c", [P, MC2], F32, kind="ExternalInput")
    b3c = nc.dram_tensor("b3c", [P, MC3], F32, kind="ExternalInput")
    outT = nc.dram_tensor("outT", [OUT_DIM, E], F32, kind="ExternalOutput")

    # --- internal DRAM projection tables (bf16 rows, gathered per bond)
    A1d = nc.dram_tensor("A1d", [LSITE, H1], BF16, kind="Internal")
    Astd = nc.dram_tensor("Astd", [N_GRAPHS, H1], BF16, kind="Internal")
    A2ch = []
    for c, (_base, rows) in enumerate(chunk_meta):
        A2ch.append(nc.dram_tensor(f"A2d{c}", [rows, H1], BF16, kind="Internal"))

    with SplitDrainTileContext(nc) as tc:
        with (
            tc.tile_pool(name="const", bufs=1) as constp,
            tc.tile_pool(name="wts", bufs=1) as wp,
            tc.tile_pool(name="idx", bufs=1) as idxp,
            tc.tile_pool(name="pstage", bufs=6) as pstage,
            tc.tile_pool(name="aout", bufs=6) as aoutp,
            tc.tile_pool(name="gath1", bufs=4) as g1p,
            tc.tile_pool(name="gath2", bufs=2) as g2p,
            tc.tile_pool(name="ssum", bufs=2) as ssump,
            tc.tile_pool(name="s01p", bufs=1) as s01p,
            tc.tile_pool(name="xT", bufs=3) as xp,
            tc.tile_pool(name="sT", bufs=2) as stp,
            tc.tile_pool(name="acts", bufs=1) as hp,
            tc.tile_pool(name="psmm", bufs=6, space="PSUM") as psmm,
            tc.tile_pool(name="psx", bufs=2, space="PSUM") as psx,
        ):
            # ---- startup loads, ordered so the first A1 matmul can fire ASAP
            ident_bf = constp.tile([P, P], BF16)
            make_identity(nc, ident_bf[:])

            w1s_sb = wp.tile([P, 4, H1], BF16)
            nc.gpsimd.dma_start(w1s_sb[:, 0, :], w1s[:, 0, :])
            nc.gpsimd.dma_start(w1s_sb[:, 1, :], w1s[:, 1, :])

            b1sb = constp.tile([P, MC1], F32)
            b2sb = constp.tile([P, MC2], F32)
            b3sb = constp.tile([P, MC3], F32)
            nc.scalar.dma_start(b1sb[:], b1c[:, :])
            nc.scalar.dma_start(b2sb[:], b2c[:, :])
            nc.scalar.dma_start(b3sb[:], b3c[:, :])

            cast_par = [0]  # alternator for PSUM->SBUF cast engine

            def emit_group(src_dram, src_col0, stages, dsts):
                """Project 512 table rows: stages = list of (row_slice, kidx)
                reading src_dram[row_slice, src_col0:src_col0+512] through
                w1s_sb[:, kidx, :] (or a direct weight AP), accumulating, then
                write bf16 rows to every (dram, row_offset) in dsts."""
                sts = []
                for (rs, _k) in stages:
                    st = pstage.tile([P, 4 * P], BF16, tag=f"st{len(sts)}")
                    nc.sync.dma_start(st[:], src_dram[rs, src_col0:src_col0 + 4 * P])
                    sts.append(st)
                for i in range(4):
                    ao = aoutp.tile([P, H1], BF16, tag="ao")
                    for h in range(2):
                        ps = psmm.tile([P, T], F32, tag="psmm")
                        hs = slice(h * 512, (h + 1) * 512)
                        for si, (st, (_rs, k)) in enumerate(zip(sts, stages)):
                            nc.tensor.matmul(
                                ps[:], st[:, i * P:(i + 1) * P],
                                w1s_sb[:, k, hs] if k >= 0 else w1st_sb[:, hs],
                                start=(si == 0), stop=(si == len(sts) - 1),
                            )
                        if (cast_par[0] + i + h) % 2 == 0:
                            nc.scalar.copy(ao[:, hs], ps[:])
                        else:
                            nc.vector.tensor_copy(ao[:, hs], ps[:])
                    cast_par[0] ^= 1
                    for (dst, roff) in dsts:
                        nc.sync.dma_start(dst[roff + i * P: roff + (i + 1) * P, :], ao[:])

            # ---- A1 (local idx1 range) through W1 rows 0:256
            for g in range(nloc // 4):
                emit_group(sitesTloc, g * GROUP,
                           [(slice(0, P), 0), (slice(P, 2 * P), 1)],
                           [(A1d, g * GROUP)])

            # ---- Ast (graph states) through W1 rows 768:896
            w1st_sb = wp.tile([P, H1], BF16)
            nc.gpsimd.dma_start(w1st_sb[:], w1st[:, :])
            emit_group(statesT, 0, [(slice(0, P), -1)], [(Astd, 0)])

            # ---- remaining weight loads (needed from tile 0 of the main loop)
            nc.gpsimd.dma_start(w1s_sb[:, 2, :], w1s[:, 2, :])
            nc.gpsimd.dma_start(w1s_sb[:, 3, :], w1s[:, 3, :])
            w1sb = wp.tile([P, KCB * MC1 * P], BF16)
            w2sb = wp.tile([P, KC2 * MC2 * P], BF16)
            w3sb = wp.tile([P, KC3 * MC3 * P], BF16)
            nc.gpsimd.dma_start(w1sb[:], w1c[:, :])
            nc.scalar.dma_start(w2sb[:], w2c[:, :])
            nc.scalar.dma_start(w3sb[:], w3c[:, :])

            idx1sb = idxp.tile([P, n_tiles * SUB], I32)
            idx2sb = idxp.tile([P, n_tiles * SUB], I32)
            g2bsb = idxp.tile([P, n_tiles * SUB], I32)
            nc.sync.dma_start(idx1sb[:], idx1[:, :])
            nc.sync.dma_start(idx2sb[:], idx2[:, :])
            nc.sync.dma_start(g2bsb[:], g2b[:, :])

            def emit_a2_group(g):
                dsts = []
                for c, (base, rows) in enumerate(chunk_meta):
                    off = g * GROUP - base
                    if 0 <= off and off + GROUP <= rows:
                        dsts.append((A2ch[c], off))
                assert dsts, f"A2 group {g} maps to no chunk"
                emit_group(sitesT2, g * GROUP,
                           [(slice(0, P), 2), (slice(P, 2 * P), 3)], dsts)

            PREF = 3

            def issue_g1(t, a1g, asg):
                for j in range(SUB):
                    cj = t * SUB + j
                    nc.gpsimd.indirect_dma_start(
                        out=a1g[:, j, :], out_offset=None, in_=A1d[:],
                        in_offset=bass.IndirectOffsetOnAxis(
                            ap=idx1sb[:, cj:cj + 1], axis=0),
                    )
                    nc.gpsimd.indirect_dma_start(
                        out=asg[:, j, :], out_offset=None, in_=Astd[:],
                        in_offset=bass.IndirectOffsetOnAxis(
                            ap=g2bsb[:, cj:cj + 1], axis=0),
                    )

            def issue_g2(t, a2g):
                src = A2ch[chunk_of[t]]
                for j in range(SUB):
                    cj = t * SUB + j
                    nc.gpsimd.indirect_dma_start(
                        out=a2g[:, j, :], out_offset=None, in_=src[:],
                        in_offset=bass.IndirectOffsetOnAxis(
                            ap=idx2sb[:, cj:cj + 1], axis=0),
                    )

            # prefetch A1/Ast gather rows for the first tiles; they overlap
            # the pre-main A2 chunk projections below.
            pre_g = {}
            for t in range(min(PREF, n_tiles)):
                a1g = g1p.tile([P, SUB, H1], BF16, tag="a1g")
                asg = g1p.tile([P, SUB, H1], BF16, tag="asg")
                issue_g1(t, a1g, asg)
                pre_g[t] = (a1g, asg)

            for g in sched_pre:
                emit_a2_group(g)

            # ================= main loop =================
            for t in range(n_tiles):
                # ---- gather projection rows: [P, SUB, H1] bf16
                if t in pre_g:
                    a1g, asg = pre_g.pop(t)
                else:
                    a1g = g1p.tile([P, SUB, H1], BF16, tag="a1g")
                    asg = g1p.tile([P, SUB, H1], BF16, tag="asg")
                    issue_g1(t, a1g, asg)
                a2g = g2p.tile([P, SUB, H1], BF16, tag="a2g")
                issue_g2(t, a2g)
                # ---- sum the three projections (still bond-major)
                s01 = s01p.tile([P, SUB, H1], BF16, tag="s01")
                ssum = ssump.tile([P, SUB, H1], BF16, tag="ssum")
                nc.vector.tensor_add(s01[:], a1g[:], a2g[:])
                nc.vector.tensor_add(ssum[:], s01[:], asg[:])

                # bonds arrive pre-transposed from the host: cast-DMA chunks
                xb = []
                for c in range(KCB):
                    xsb = xp.tile([P, T], BF16, tag=f"xTb{c}")
                    nc.sync.dma_start(
                        xsb[:], bondsT[c * P:(c + 1) * P, t * T:(t + 1) * T])
                    xb.append(xsb)

                # ---- layer 1: bond-block matmul + transposed projection sum
                h1T = []
                for m in range(MC1):
                    ps = psmm.tile([P, T], F32, tag="psmm")
                    for k in range(KCB):
                        nc.tensor.matmul(
                            ps[:],
                            w1sb[:, (k * MC1 + m) * P:(k * MC1 + m + 1) * P],
                            xb[k][:],
                            start=(k == 0), stop=(k == KCB - 1),
                        )
                    pst = psx.tile([P, T], BF16, tag="psx")
                    for j in range(SUB):
                        nc.tensor.transpose(
                            pst[:, j * P:(j + 1) * P],
                            ssum[:, j, m * P:(m + 1) * P],
                            ident_bf[:],
                        )
                    sT = stp.tile([P, T], BF16, tag="sT")
                    nc.vector.tensor_copy(sT[:], pst[:])
                    pre = stp.tile([P, T], BF16, tag="pre")
                    nc.vector.tensor_add(pre[:], ps[:], sT[:])
                    hsb = hp.tile([P, T], BF16, tag=f"h1T{m}")
                    nc.scalar.activation(
                        hsb[:], pre[:], mybir.ActivationFunctionType.Relu,
                        bias=b1sb[:, m:m + 1],
                    )
                    h1T.append(hsb)

                # ---- layer 2
                h2T = []
                for m in range(MC2):
                    ps = psmm.tile([P, T], F32, tag="psmm")
                    for k in range(KC2):
                        nc.tensor.matmul(
                            ps[:],
                            w2sb[:, (k * MC2 + m) * P:(k * MC2 + m + 1) * P],
                            h1T[k][:],
                            start=(k == 0), stop=(k == KC2 - 1),
                        )
                    hsb = hp.tile([P, T], BF16, tag=f"h2T{m}")
                    nc.scalar.activation(
                        hsb[:], ps[:], mybir.ActivationFunctionType.Relu,
                        bias=b2sb[:, m:m + 1],
                    )
                    h2T.append(hsb)

                # ---- layer 3
                oT = []
                for m in range(MC3):
                    ps = psmm.tile([P, T], F32, tag="psmm")
                    for k in range(KC3):
                        nc.tensor.matmul(
                            ps[:],
                            w3sb[:, (k * MC3 + m) * P:(k * MC3 + m + 1) * P],
                            h2T[k][:],
                            start=(k == 0), stop=(k == KC3 - 1),
                        )
                    hsb = hp.tile([P, T], F32R, tag=f"oT{m}")
                    nc.scalar.activation(
                        hsb[:], ps[:], mybir.ActivationFunctionType.Relu,
                        bias=b3sb[:, m:m + 1],
                    )
                    oT.append(hsb)

                # ---- store transposed output; host un-transposes
                for c in range(MC3):
                    nc.sync.dma_start(
                        outT[c * P:(c + 1) * P, t * T:(t + 1) * T],
                        oT[c][:].bitcast(F32),
                    )

                # ---- just-in-time A2 chunk projections for upcoming tiles
                for g in sched_main[t]:
                    emit_a2_group(g)

    _legalize_waits(nc)
    return nc


def _prep_shared(W1, b1, W2, b2, W3, b3):
    BF = ml_dtypes.bfloat16
    W1 = np.asarray(W1, dtype=np.float32)

    def chunk_w(W, KC, MC):
        # [KC*P, MC*P] -> [P, KC*MC*P] with w[p, (k*MC+m)*P+j] = W[k*P+p, m*P+j]
        return np.ascontiguousarray(
            W.reshape(KC, P, MC, P).transpose(1, 0, 2, 3).reshape(P, KC * MC * P)
        ).astype(BF)

    def chunk_b(b, MC):
        return np.ascontiguousarray(np.asarray(b).reshape(MC, P).T).astype(
            np.float32, copy=False)

    return {
        "w1s": np.ascontiguousarray(
            W1[0:512].reshape(4, P, H1).transpose(1, 0, 2)).astype(BF),
        "w1st": np.ascontiguousarray(W1[768:896]).astype(BF),
        "w1c": chunk_w(W1[512:768], KCB, MC1),
        "w2c": chunk_w(np.asarray(W2, dtype=np.float32), KC2, MC2),
        "w3c": chunk_w(np.asarray(W3, dtype=np.float32), KC3, MC3),
        "b1c": chunk_b(b1, MC1),
        "b2c": chunk_b(b2, MC2),
        "b3c": chunk_b(b3, MC3),
    }


def _wrap_idx(raw: np.ndarray) -> np.ndarray:
    # [E_core] -> [P, n_tiles*SUB] with idx[p, q] = raw[q*P + p]
    n = raw.shape[0] // P
    return np.ascontiguousarray(raw.reshape(n, P).T).astype(np.int32, copy=False)


_BUILT = {}


def _get_bass(key, *args) -> bass.Bass:
    if key not in _BUILT:
        _BUILT[key] = build_bass(*args)
    return _BUILT[key]


def prepare(sites, bonds, states, indices1, indices2, graph_to_bonds,
            W1, b1, W2, b2, W3, b3):
    """Shard + reformat full inputs. Returns (nc, in_maps, perm, n_tiles)."""
    i1 = np.asarray(indices1).astype(np.int64, copy=False)
    i2 = np.asarray(indices2).astype(np.int64, copy=False)
    gb = np.asarray(graph_to_bonds).astype(np.int64, copy=False)
    bonds = np.asarray(bonds, dtype=np.float32)
    n_bonds = bonds.shape[0]
    assert n_bonds == N_BONDS

    # shard by idx1 range, then sort each shard by idx2 so A2 gathers sweep
    # the rank-ordered dedup table monotonically
    perm0 = np.argsort(i1, kind="stable")
    starts = [c * E_SHARD for c in range(N_CORES)]
    i1_sorted = i1[perm0]
    los = [int(i1_sorted[s]) for s in starts]
    his = [int(i1_sorted[s + E_SHARD - 1]) for s in starts]
    sizes = [hi - lo + 1 for lo, hi in zip(los, his)]
    nloc = max(20, 4 * (-(-max(sizes) // (4 * P))))
    LSITE = nloc * P

    perm = np.empty_like(perm0)
    for c in range(N_CORES):
        sl = slice(starts[c], starts[c] + E_SHARD)
        sub = perm0[sl]
        order = np.argsort(i2[sub], kind="stable")
        perm[sl] = sub[order]

    i1s, i2s, gbs = i1[perm], i2[perm], gb[perm]
    bondsT_s = np.ascontiguousarray(bonds[perm].T)  # [256, n_bonds]

    n_tiles = max(TILES_PER_CORE, -(-E_SHARD // T))
    e_core = n_tiles * T

    BF = ml_dtypes.bfloat16
    sitesT_bf = np.asarray(sites, dtype=np.float32).T.astype(BF)  # [256, N_SITES]
    statesT_bf = np.ascontiguousarray(
        np.asarray(states, dtype=np.float32).T).astype(BF)
    bondsT_bf = bondsT_s.astype(BF)

    # dedup idx2 per core; ranks are non-decreasing with steps in {0, 1}
    refs, ranks_pad = [], []
    for c in range(N_CORES):
        sl = slice(starts[c], starts[c] + E_SHARD)
        r = np.unique(i2s[sl])
        refs.append(r)
        rk = np.searchsorted(r, i2s[sl])
        rp = np.concatenate([rk, np.full(e_core - E_SHARD, rk[-1], dtype=rk.dtype)])
        ranks_pad.append(rp)
    nch2g = max(28, max(-(-len(r) // GROUP) for r in refs))
    LS2 = nch2g * GROUP
    NCH = -(-LS2 // CHUNK_STEP)
    chunk_meta = []
    for c in range(NCH):
        base = max(0, c * CHUNK_STEP - CHUNK_MARGIN)
        end = min(LS2, c * CHUNK_STEP - CHUNK_MARGIN + CHUNK_SPAN)
        chunk_meta.append((base, end - base))
    chunk_meta = tuple(chunk_meta)

    # shared (across cores) per-tile chunk choice
    chunk_of = []
    prev = 0
    for t in range(n_tiles):
        lo_t = min(int(rp[t * T]) for rp in ranks_pad)
        hi_t = max(int(rp[t * T + T - 1]) for rp in ranks_pad)
        pick = None
        for c in range(prev, NCH):
            base, rows = chunk_meta[c]
            if base <= lo_t and hi_t < base + rows:
                pick = c
                break
        assert pick is not None, (t, lo_t, hi_t, chunk_meta)
        chunk_of.append(pick)
        prev = pick
    chunk_of = tuple(chunk_of)

    # group emission schedule: each A2 group must be written before the first
    # tile that gathers from any chunk containing it
    first_tile = {}
    for t, c in enumerate(chunk_of):
        first_tile.setdefault(c, t)
    # chunks never picked: deadline of the next picked chunk (or end)
    d = [first_tile.get(c, n_tiles) for c in range(NCH)]
    for c in range(NCH - 2, -1, -1):
        d[c] = min(d[c], d[c + 1])
    tgt = {g: [c for c in range(NCH)
               if chunk_meta[c][0] <= g * GROUP
               and (g + 1) * GROUP <= chunk_meta[c][0] + chunk_meta[c][1]]
           for g in range(nch2g)}
    sched_pre, sched_main = [], [[] for _ in range(n_tiles)]
    for g in range(nch2g):
        deadline = min(d[c] for c in tgt[g])
        slot = deadline - SCHED_CUSHION
        if slot < 0:
            sched_pre.append(g)
        else:
            sched_main[min(slot, n_tiles - 1)].append(g)
    sched_pre = tuple(sched_pre)
    sched_main = tuple(tuple(s) for s in sched_main)

    shared = _prep_shared(W1, b1, W2, b2, W3, b3)
    in_maps = []
    for c in range(N_CORES):
        lo = los[c]
        sl = slice(starts[c], starts[c] + E_SHARD)
        stl = np.zeros((SITE_LEN, LSITE), dtype=BF)
        avail = min(LSITE, N_SITES - lo)
        stl[:, :avail] = sitesT_bf[:, lo:lo + avail]

        st2 = np.zeros((SITE_LEN, LS2), dtype=BF)
        st2[:, :len(refs[c])] = sitesT_bf[:, refs[c]]

        i1_loc = np.zeros(e_core, dtype=np.int64)
        i1_loc[:E_SHARD] = i1s[sl] - lo
        # idx2: rank adjusted to be chunk-relative per tile
        i2_adj = ranks_pad[c].astype(np.int64, copy=True)
        for t in range(n_tiles):
            base = chunk_meta[chunk_of[t]][0]
            blk = slice(t * T, (t + 1) * T)
            i2_adj[blk] -= base
            assert i2_adj[blk].min() >= 0
            assert i2_adj[blk].max() < chunk_meta[chunk_of[t]][1]
        gb_pad = np.zeros(e_core, dtype=np.int64)
        gb_pad[:E_SHARD] = gbs[sl]
        bT = np.zeros((BOND_LEN, e_core), dtype=BF)
        bT[:, :E_SHARD] = bondsT_bf[:, sl]

        m = {
            "sitesT2": st2,
            "sitesTloc": stl,
            "statesT": statesT_bf,
            "bondsT": bT,
            "idx1": _wrap_idx(i1_loc),
            "idx2": _wrap_idx(i2_adj),
            "g2b": _wrap_idx(gb_pad),
        }
        m.update(shared)
        in_maps.append(m)

    key = (n_tiles, nloc, nch2g, chunk_meta, chunk_of, sched_pre, sched_main)
    nc = _get_bass(key, n_tiles, nloc, nch2g, chunk_meta, chunk_of,
                   sched_pre, sched_main)
    return nc, in_maps, perm, n_tiles


def kernel(sites, bonds, states, indices1, indices2, graph_to_bonds,
           W1, b1, W2, b2, W3, b3):
    nc, in_maps, perm, n_tiles = prepare(
        sites, bonds, states, indices1, indices2, graph_to_bonds,
        W1, b1, W2, b2, W3, b3)
    res = run_bass_kernel_spmd(nc, in_maps, core_ids=list(range(N_CORES)))
    out = np.empty((N_BONDS, OUT_DIM), dtype=np.float32)
    for c in range(N_CORES):
        sl = slice(c * E_SHARD, (c + 1) * E_SHARD)
        out[perm[sl]] = res.results[c]["outT"][:, :E_SHARD].T
    return out


# revision 14
# speedup vs baseline: 1.0332x; 1.0332x over previous
"""Trainium2 Bass kernel for BondUpdate GNN message passing.

Computes, for each bond e:
    x = concat(sites[idx1[e]], sites[idx2[e]], bonds[e], states[g2b[e]])  # [896]
    out[e] = relu(relu(relu(x @ W1 + b1) @ W2 + b2) @ W3 + b3)           # [256]

Strategy (v3): the 20000 sites are referenced ~400k times via idx1/idx2, and
512 graph states ~200k times, so the W1 blocks that multiply site/state
features are precomputed per core into DRAM projection tables:
    A1 = sites @ W1[0:256]      (only the local idx1 range)
    A2 = sites @ W1[256:512]    (deduped idx2 sites for this core)
    Ast = states @ W1[768:896]
Per bond, layer 1 then reduces to a 256-wide matmul on the bond features plus
a gather+sum of three projection rows, transposed into feature-major via the
PE. Bonds are sharded across 8 cores by idx1 range (so A1 is small), and
WITHIN each core sorted by idx2, so each tile's A2 rows form a narrow window
of the (rank-ordered) dedup table. The A2 table is then built as overlapping
2048-rank chunks (span 3584 rows) in separate DRAM tensors, and chunk
projection work is interleaved INTO the main loop just-in-time: the tensor
engine never sits idle waiting for the whole A2 table (the v2 kernel lost
~75us to precompute-phase stalls). Chunk choice per tile is shared across
cores (SPMD single program): the +-512-rank margin absorbs cross-core rank
fluctuations.

Activations stay transposed in SBUF (features on partitions, bonds on free
dim) so the three matmul layers chain without intermediate transposes.
Matmul operands are bf16 (PSUM accumulation fp32, biases+relu applied in
fp32, final output stage f32r so values are not re-rounded).
"""
import sys

if "/opt/trn_rl_repo" not in sys.path:
    sys.path.insert(0, "/opt/trn_rl_repo")

import ml_dtypes
import numpy as np

import concourse.bass as bass
import concourse.mybir as mybir
import concourse.tile as tile
from concourse.bass_utils import run_bass_kernel_spmd
from concourse.masks import make_identity
from concourse.vector_clock import ScopedClock

F32 = mybir.dt.float32
F32R = mybir.dt.float32r
BF16 = mybir.dt.bfloat16
I32 = mybir.dt.int32

P = 128            # partitions
T = 512            # bonds per tile
SUB = T // P       # 128-bond subtiles per tile

N_SITES = 20000
N_GRAPHS = 512
SITE_LEN = 256
BOND_LEN = 256
STATE_LEN = 128
H1 = 1024
H2 = 1024
OUT_DIM = 256

KCB, MC1 = BOND_LEN // P, H1 // P  # 2, 8   (bond block of W1)
KC2, MC2 = H1 // P, H2 // P        # 8, 8
KC3, MC3 = H2 // P, OUT_DIM // P   # 8, 2

N_CORES = 8
N_BONDS = 200000
E_SHARD = N_BONDS // N_CORES       # 25000 bonds per core
TILES_PER_CORE = 49                # 49*512 = 25088 >= 25000
GROUP = 512                        # table rows per projection group
CHUNK_STEP = 2048                  # rank stride between A2 chunks
CHUNK_MARGIN = 512                 # low-side rank margin per chunk
CHUNK_SPAN = 3584                  # rows covered by one chunk tensor
SCHED_CUSHION = 3                  # emit chunk groups >= this many tiles early

EVSEM_WAIT_CAP = 2  # InstEventSemaphore holds 2 waits; every other inst 1


def _legalize_waits(nc: bass.Bass):
    """Spill sync waits beyond the per-instruction capacity onto standalone
    InstEventSemaphore instructions inserted just before the offender.
    Walrus here rejects instructions with more waits than the ISA slots."""
    n_spilled = 0
    for f in nc.m.functions:
        for bb in f.blocks:
            il = bb.instructions
            i = 0
            while i < len(il):
                inst = il[i]
                si = inst.sync_info
                waits = list(si.on_wait) if si and si.on_wait else []
                cap = (
                    EVSEM_WAIT_CAP
                    if isinstance(inst, mybir.InstEventSemaphore)
                    else 1
                )
                if len(waits) > cap:
                    keep = waits[-cap:]
                    spill = waits[:-cap]
                    si.on_wait = keep
                    evs = []
                    for j in range(0, len(spill), EVSEM_WAIT_CAP):
                        ev = mybir.InstEventSemaphore(
                            name=nc.get_next_instruction_name(),
                            ins=[],
                            outs=[],
                            sync_info=mybir.SyncInfo(
                                on_wait=spill[j:j + EVSEM_WAIT_CAP],
                                on_update=[],
                            ),
                        )
                        ev.engine = inst.engine
                        nc.register_instruction(ev)
                        evs.append(ev)
                    il[i:i] = evs
                    i += len(evs)
                    n_spilled += len(spill)
                i += 1
    return n_spilled


class SplitDrainTileContext(tile.TileContext):
    """TileContext whose kernel-tail drain also respects the wait cap."""

    def _drain_and_barrier(self, tick_clock, wait_clock):
        nc = self.nc
        drain_inst = nc.sync.drain()
        wait_clock.add_sem_waits(
            drain_inst.ins, ScopedClock({None: tick_clock.global_clock})
        )
        si = drain_inst.ins.sync_info
        waits = list(si.on_wait or [])
        if len(waits) > 1:
            si.on_wait = []
            id2sem = {s.num: s for s in self.sems.allocated().values()}
            for w in waits:
                assert w.wait_mode == "sem-ge-imm", w
                nc.sync.wait_ge(id2sem[w.id], w.wait_value)
        nc.all_engine_barrier()
        assert self.sems is not None
        popped = nc._tile_sem_poison_stack.pop()
        assert popped is self._sem_poison
        nc.clear_and_free_semaphores(list(self.sems.allocated().values()))
        nc.all_engine_barrier()


def build_bass(n_tiles: int, nloc: int, nch2g: int,
               chunk_meta: tuple, chunk_of: tuple,
               pre0: tuple, fill: tuple,
               stage_at: tuple, compute_at: tuple) -> bass.Bass:
    """Per-core Bass program.

    chunk_meta: tuple of (base_row, n_rows) per A2 chunk tensor.
    chunk_of:   per tile, which chunk its idx2 gathers read.
    pre0:       A2 group ids computed before the gather prefetches (chunk 0).
    fill:       A2 group ids computed pre-main as latency filler.
    stage_at:   per tile, A2 group ids whose staging DMAs issue before
                that tile's body.
    compute_at: per tile, A2 group ids whose matmuls+writes are emitted
                after that tile's body.
    """
    nc = bass.Bass("TRN2", target_bir_lowering=False, debug=False, num_devices=1)
    E = n_tiles * T
    LSITE = nloc * P
    LS2 = nch2g * GROUP
    NCH = len(chunk_meta)

    # --- external inputs
    sitesT2 = nc.dram_tensor("sitesT2", [SITE_LEN, LS2], BF16, kind="ExternalInput")
    sitesTloc = nc.dram_tensor("sitesTloc", [SITE_LEN, LSITE], BF16, kind="ExternalInput")
    statesT = nc.dram_tensor("statesT", [STATE_LEN, N_GRAPHS], BF16, kind="ExternalInput")
    bondsT = nc.dram_tensor("bondsT", [BOND_LEN, E], BF16, kind="ExternalInput")
    # indices pre-wrapped on host to [P, n_tiles*SUB]: idx[p, t*SUB+j] = raw[t*T + j*P + p]
    idx1 = nc.dram_tensor("idx1", [P, n_tiles * SUB], I32, kind="ExternalInput")
    idx2 = nc.dram_tensor("idx2", [P, n_tiles * SUB], I32, kind="ExternalInput")
    g2b = nc.dram_tensor("g2b", [P, n_tiles * SUB], I32, kind="ExternalInput")
    # W1 site block rows 0:512 as [p, k, h] = W1[k*128+p, h], k=0..3 (bf16)
    w1s = nc.dram_tensor("w1s", [P, 4, H1], BF16, kind="ExternalInput")
    # W1 state block rows 768:896: [p, h] = W1[768+p, h]
    w1st = nc.dram_tensor("w1st", [P, H1], BF16, kind="ExternalInput")
    # W1 bond block rows 512:768 chunked: w1c[p, (k*MC1+m)*P+j] = W1[512+k*P+p, m*P+j]
    w1c = nc.dram_tensor("w1c", [P, KCB * MC1 * P], BF16, kind="ExternalInput")
    w2c = nc.dram_tensor("w2c", [P, KC2 * MC2 * P], BF16, kind="ExternalInput")
    w3c = nc.dram_tensor("w3c", [P, KC3 * MC3 * P], BF16, kind="ExternalInput")
    # biases pre-wrapped: bXc[p, m] = bX[m*P+p]
    b1c = nc.dram_tensor("b1c", [P, MC1], F32, kind="ExternalInput")
    b2c = nc.dram_tensor("b2c", [P, MC2], F32, kind="ExternalInput")
    b3c = nc.dram_tensor("b3c", [P, MC3], F32, kind="ExternalInput")
    outT = nc.dram_tensor("outT", [OUT_DIM, E], F32, kind="ExternalOutput")

    # --- internal DRAM projection tables (bf16 rows, gathered per bond)
    A1d = nc.dram_tensor("A1d", [LSITE, H1], BF16, kind="Internal")
    Astd = nc.dram_tensor("Astd", [N_GRAPHS, H1], BF16, kind="Internal")
    A2ch = []
    for c, (_base, rows) in enumerate(chunk_meta):
        A2ch.append(nc.dram_tensor(f"A2d{c}", [rows, H1], BF16, kind="Internal"))

    with SplitDrainTileContext(nc) as tc:
        with (
            tc.tile_pool(name="const", bufs=1) as constp,
            tc.tile_pool(name="wts", bufs=1) as wp,
            tc.tile_pool(name="idx", bufs=1) as idxp,
            tc.tile_pool(name="pstage", bufs=6) as pstage,
            tc.tile_pool(name="aout", bufs=6) as aoutp,
            tc.tile_pool(name="gath1", bufs=4) as g1p,
            tc.tile_pool(name="gath2", bufs=2) as g2p,
            tc.tile_pool(name="ssum", bufs=2) as ssump,
            tc.tile_pool(name="s01p", bufs=1) as s01p,
            tc.tile_pool(name="xT", bufs=3) as xp,
            tc.tile_pool(name="sT", bufs=2) as stp,
            tc.tile_pool(name="acts", bufs=1) as hp,
            tc.tile_pool(name="psmm", bufs=6, space="PSUM") as psmm,
            tc.tile_pool(name="psx", bufs=2, space="PSUM") as psx,
        ):
            # ---- startup loads: w1s k2/k3 first (A2 chunk-0 groups run
            # first), idx on the scalar queue so sync is free for staging
            ident_bf = constp.tile([P, P], BF16)
            make_identity(nc, ident_bf[:])

            w1s_sb = wp.tile([P, 4, H1], BF16)
            for k in (2, 3, 0, 1):
                nc.gpsimd.dma_start(w1s_sb[:, k, :], w1s[:, k, :])
            w1st_sb = wp.tile([P, H1], BF16)
            nc.gpsimd.dma_start(w1st_sb[:], w1st[:, :])

            b1sb = constp.tile([P, MC1], F32)
            b2sb = constp.tile([P, MC2], F32)
            b3sb = constp.tile([P, MC3], F32)
            nc.scalar.dma_start(b1sb[:], b1c[:, :])
            nc.scalar.dma_start(b2sb[:], b2c[:, :])
            nc.scalar.dma_start(b3sb[:], b3c[:, :])

            idx1sb = idxp.tile([P, n_tiles * SUB], I32)
            idx2sb = idxp.tile([P, n_tiles * SUB], I32)
            g2bsb = idxp.tile([P, n_tiles * SUB], I32)
            nc.scalar.dma_start(idx2sb[:], idx2[:, :])
            nc.scalar.dma_start(idx1sb[:], idx1[:, :])
            nc.scalar.dma_start(g2bsb[:], g2b[:, :])

            w1sb = wp.tile([P, KCB * MC1 * P], BF16)
            w2sb = wp.tile([P, KC2 * MC2 * P], BF16)
            w3sb = wp.tile([P, KC3 * MC3 * P], BF16)

            def stage_unit(src_dram, src_col0, nst):
                sts = []
                for s in range(nst):
                    st = pstage.tile([P, 4 * P], BF16, tag=f"st{s}")
                    nc.sync.dma_start(
                        st[:], src_dram[s * P:(s + 1) * P,
                                        src_col0:src_col0 + 4 * P])
                    sts.append(st)
                return sts

            def compute_unit(sts, ks, dsts):
                """Project 512 staged table rows through w1s chunk(s) ks,
                write bf16 rows to every (dram, row_offset) in dsts. PSUM
                drains via the vector engine (idle at group-emission points)."""
                for i in range(4):
                    ao = aoutp.tile([P, H1], BF16, tag="ao")
                    for h in range(2):
                        ps = psmm.tile([P, T], F32, tag="psmm")
                        hs = slice(h * 512, (h + 1) * 512)
                        for si, (st, k) in enumerate(zip(sts, ks)):
                            nc.tensor.matmul(
                                ps[:], st[:, i * P:(i + 1) * P],
                                w1s_sb[:, k, hs] if k >= 0 else w1st_sb[:, hs],
                                start=(si == 0), stop=(si == len(sts) - 1),
                            )
                        nc.vector.tensor_copy(ao[:, hs], ps[:])
                    for (dst, roff) in dsts:
                        nc.sync.dma_start(dst[roff + i * P: roff + (i + 1) * P, :], ao[:])

            def a2_dsts(g):
                dsts = []
                for c, (base, rows) in enumerate(chunk_meta):
                    off = g * GROUP - base
                    if 0 <= off and off + GROUP <= rows:
                        dsts.append((A2ch[c], off))
                assert dsts, f"A2 group {g} maps to no chunk"
                return dsts

            def stage_a2(g):
                return stage_unit(sitesT2, g * GROUP, 2)

            def compute_a2(g, sts):
                compute_unit(sts, (2, 3), a2_dsts(g))

            staged = {}

            PREF = 3

            def issue_g1(t, a1g, asg):
                for j in range(SUB):
                    cj = t * SUB + j
                    nc.gpsimd.indirect_dma_start(
                        out=a1g[:, j, :], out_offset=None, in_=A1d[:],
                        in_offset=bass.IndirectOffsetOnAxis(
                            ap=idx1sb[:, cj:cj + 1], axis=0),
                    )
                    nc.gpsimd.indirect_dma_start(
                        out=asg[:, j, :], out_offset=None, in_=Astd[:],
                        in_offset=bass.IndirectOffsetOnAxis(
                            ap=g2bsb[:, cj:cj + 1], axis=0),
                    )

            def issue_g2(t, a2g):
                src = A2ch[chunk_of[t]]
                for j in range(SUB):
                    cj = t * SUB + j
                    nc.gpsimd.indirect_dma_start(
                        out=a2g[:, j, :], out_offset=None, in_=src[:],
                        in_offset=bass.IndirectOffsetOnAxis(
                            ap=idx2sb[:, cj:cj + 1], axis=0),
                    )

            # ---- pre-main pipeline: chunk-0 groups, A1, Ast, then chunk-1
            # filler groups that keep the PE busy while the first tiles'
            # gathers+adds land. Staging runs 4 units ahead on sync.
            units = ([("a2", g) for g in pre0]
                     + [("a1", g) for g in range(nloc // 4)]
                     + [("ast", 0)]
                     + [("a2", g) for g in fill])

            def stage_pre(u):
                kind, g = u
                if kind == "a2":
                    staged[g] = stage_a2(g)
                elif kind == "a1":
                    staged[("a1", g)] = stage_unit(sitesTloc, g * GROUP, 2)
                else:
                    staged[("ast", 0)] = stage_unit(statesT, 0, 1)

            pre_g = {}
            pre_a2 = {}
            LOOKAHEAD = 4
            for u in units[:LOOKAHEAD]:
                stage_pre(u)
            n_pre0 = len(pre0)
            for ui, u in enumerate(units):
                kind, g = u
                if kind == "a2":
                    compute_a2(g, staged.pop(g))
                elif kind == "a1":
                    compute_unit(staged.pop(("a1", g)), (0, 1),
                                 [(A1d, g * GROUP)])
                else:
                    compute_unit(staged.pop(("ast", 0)), (-1,), [(Astd, 0)])
                if ui + LOOKAHEAD < len(units):
                    stage_pre(units[ui + LOOKAHEAD])
                if ui == n_pre0 - 1:
                    # chunk 0 written: prefetch the first tiles' idx2 gathers
                    for t in range(min(2, n_tiles)):
                        a2g = g2p.tile([P, SUB, H1], BF16, tag="a2g")
                        issue_g2(t, a2g)
                        pre_a2[t] = a2g
                    # weight loads for the main loop ride behind on gpsimd
                    nc.gpsimd.dma_start(w1sb[:], w1c[:, :])
                if ui == n_pre0 + nloc // 4:
                    # A1 + Ast written: prefetch the first tiles' row gathers
                    for t in range(min(PREF, n_tiles)):
                        a1g = g1p.tile([P, SUB, H1], BF16, tag="a1g")
                        asg = g1p.tile([P, SUB, H1], BF16, tag="asg")
                        issue_g1(t, a1g, asg)
                        pre_g[t] = (a1g, asg)
                    nc.scalar.dma_start(w2sb[:], w2c[:, :])
                    nc.scalar.dma_start(w3sb[:], w3c[:, :])

            # ================= main loop =================
            for t in range(n_tiles):
                # staging DMAs for groups computed after the NEXT tile
                for g in stage_at[t]:
                    staged[g] = stage_a2(g)
                # ---- gather projection rows: [P, SUB, H1] bf16
                if t in pre_g:
                    a1g, asg = pre_g.pop(t)
                else:
                    a1g = g1p.tile([P, SUB, H1], BF16, tag="a1g")
                    asg = g1p.tile([P, SUB, H1], BF16, tag="asg")
                    issue_g1(t, a1g, asg)
                if t in pre_a2:
                    a2g = pre_a2.pop(t)
                else:
                    a2g = g2p.tile([P, SUB, H1], BF16, tag="a2g")
                    issue_g2(t, a2g)
                # ---- sum the three projections (still bond-major)
                s01 = s01p.tile([P, SUB, H1], BF16, tag="s01")
                ssum = ssump.tile([P, SUB, H1], BF16, tag="ssum")
                nc.vector.tensor_add(s01[:], a1g[:], a2g[:])
                nc.vector.tensor_add(ssum[:], s01[:], asg[:])

                # bonds arrive pre-transposed from the host: cast-DMA chunks
                xb = []
                for c in range(KCB):
                    xsb = xp.tile([P, T], BF16, tag=f"xTb{c}")
                    nc.sync.dma_start(
                        xsb[:], bondsT[c * P:(c + 1) * P, t * T:(t + 1) * T])
                    xb.append(xsb)

                # ---- layer 1: bond-block matmul + transposed projection sum
                h1T = []
                for m in range(MC1):
                    ps = psmm.tile([P, T], F32, tag="psmm")
                    for k in range(KCB):
                        nc.tensor.matmul(
                            ps[:],
                            w1sb[:, (k * MC1 + m) * P:(k * MC1 + m + 1) * P],
                            xb[k][:],
                            start=(k == 0), stop=(k == KCB - 1),
                        )
                    pst = psx.tile([P, T], BF16, tag="psx")
                    for j in range(SUB):
                        nc.tensor.transpose(
                            pst[:, j * P:(j + 1) * P],
                            ssum[:, j, m * P:(m + 1) * P],
                            ident_bf[:],
                        )
                    sT = stp.tile([P, T], BF16, tag="sT")
                    nc.vector.tensor_copy(sT[:], pst[:])
                    pre = stp.tile([P, T], BF16, tag="pre")
                    nc.vector.tensor_add(pre[:], ps[:], sT[:])
                    hsb = hp.tile([P, T], BF16, tag=f"h1T{m}")
                    nc.scalar.activation(
                        hsb[:], pre[:], mybir.ActivationFunctionType.Relu,
                        bias=b1sb[:, m:m + 1],
                    )
                    h1T.append(hsb)

                # ---- layer 2
                h2T = []
                for m in range(MC2):
                    ps = psmm.tile([P, T], F32, tag="psmm")
                    for k in range(KC2):
                        nc.tensor.matmul(
                            ps[:],
                            w2sb[:, (k * MC2 + m) * P:(k * MC2 + m + 1) * P],
                            h1T[k][:],
                            start=(k == 0), stop=(k == KC2 - 1),
                        )
                    hsb = hp.tile([P, T], BF16, tag=f"h2T{m}")
                    nc.scalar.activation(
                        hsb[:], ps[:], mybir.ActivationFunctionType.Relu,
                        bias=b2sb[:, m:m + 1],
                    )
                    h2T.append(hsb)

                # ---- layer 3
                oT = []
                for m in range(MC3):
                    ps = psmm.tile([P, T], F32, tag="psmm")
                    for k in range(KC3):
                        nc.tensor.matmul(
                            ps[:],
                            w3sb[:, (k * MC3 + m) * P:(k * MC3 + m + 1) * P],
                            h2T[k][:],
                            start=(k == 0), stop=(k == KC3 - 1),
                        )
                    hsb = hp.tile([P, T], F32R, tag=f"oT{m}")
                    nc.scalar.activation(
                        hsb[:], ps[:], mybir.ActivationFunctionType.Relu,
                        bias=b3sb[:, m:m + 1],
                    )
                    oT.append(hsb)

                # ---- store transposed output; host un-transposes
                for c in range(MC3):
                    nc.sync.dma_start(
                        outT[c * P:(c + 1) * P, t * T:(t + 1) * T],
                        oT[c][:].bitcast(F32),
                    )

                # ---- just-in-time A2 chunk projections for upcoming tiles
                for g in compute_at[t]:
                    compute_a2(g, staged.pop(g))

    _legalize_waits(nc)
    return nc


def _prep_shared(W1, b1, W2, b2, W3, b3):
    BF = ml_dtypes.bfloat16
    W1 = np.asarray(W1, dtype=np.float32)

    def chunk_w(W, KC, MC):
        # [KC*P, MC*P] -> [P, KC*MC*P] with w[p, (k*MC+m)*P+j] = W[k*P+p, m*P+j]
        return np.ascontiguousarray(
            W.reshape(KC, P, MC, P).transpose(1, 0, 2, 3).reshape(P, KC * MC * P)
        ).astype(BF)

    def chunk_b(b, MC):
        return np.ascontiguousarray(np.asarray(b).reshape(MC, P).T).astype(
            np.float32, copy=False)

    return {
        "w1s": np.ascontiguousarray(
            W1[0:512].reshape(4, P, H1).transpose(1, 0, 2)).astype(BF),
        "w1st": np.ascontiguousarray(W1[768:896]).astype(BF),
        "w1c": chunk_w(W1[512:768], KCB, MC1),
        "w2c": chunk_w(np.asarray(W2, dtype=np.float32), KC2, MC2),
        "w3c": chunk_w(np.asarray(W3, dtype=np.float32), KC3, MC3),
        "b1c": chunk_b(b1, MC1),
        "b2c": chunk_b(b2, MC2),
        "b3c": chunk_b(b3, MC3),
    }


def _wrap_idx(raw: np.ndarray) -> np.ndarray:
    # [E_core] -> [P, n_tiles*SUB] with idx[p, q] = raw[q*P + p]
    n = raw.shape[0] // P
    return np.ascontiguousarray(raw.reshape(n, P).T).astype(np.int32, copy=False)


_BUILT = {}


def _get_bass(key, *args) -> bass.Bass:
    if key not in _BUILT:
        _BUILT[key] = build_bass(*args)
    return _BUILT[key]


def prepare(sites, bonds, states, indices1, indices2, graph_to_bonds,
            W1, b1, W2, b2, W3, b3):
    """Shard + reformat full inputs. Returns (nc, in_maps, perm, n_tiles)."""
    i1 = np.asarray(indices1).astype(np.int64, copy=False)
    i2 = np.asarray(indices2).astype(np.int64, copy=False)
    gb = np.asarray(graph_to_bonds).astype(np.int64, copy=False)
    bonds = np.asarray(bonds, dtype=np.float32)
    n_bonds = bonds.shape[0]
    assert n_bonds == N_BONDS

    # shard by idx1 range, then sort each shard by idx2 so A2 gathers sweep
    # the rank-ordered dedup table monotonically
    perm0 = np.argsort(i1, kind="stable")
    starts = [c * E_SHARD for c in range(N_CORES)]
    i1_sorted = i1[perm0]
    los = [int(i1_sorted[s]) for s in starts]
    his = [int(i1_sorted[s + E_SHARD - 1]) for s in starts]
    sizes = [hi - lo + 1 for lo, hi in zip(los, his)]
    nloc = max(20, 4 * (-(-max(sizes) // (4 * P))))
    LSITE = nloc * P

    perm = np.empty_like(perm0)
    for c in range(N_CORES):
        sl = slice(starts[c], starts[c] + E_SHARD)
        sub = perm0[sl]
        order = np.argsort(i2[sub], kind="stable")
        perm[sl] = sub[order]

    i1s, i2s, gbs = i1[perm], i2[perm], gb[perm]
    bondsT_s = np.ascontiguousarray(bonds[perm].T)  # [256, n_bonds]

    n_tiles = max(TILES_PER_CORE, -(-E_SHARD // T))
    e_core = n_tiles * T

    BF = ml_dtypes.bfloat16
    sitesT_bf = np.asarray(sites, dtype=np.float32).T.astype(BF)  # [256, N_SITES]
    statesT_bf = np.ascontiguousarray(
        np.asarray(states, dtype=np.float32).T).astype(BF)
    bondsT_bf = bondsT_s.astype(BF)

    # dedup idx2 per core; ranks are non-decreasing with steps in {0, 1}
    refs, ranks_pad = [], []
    for c in range(N_CORES):
        sl = slice(starts[c], starts[c] + E_SHARD)
        r = np.unique(i2s[sl])
        refs.append(r)
        rk = np.searchsorted(r, i2s[sl])
        rp = np.concatenate([rk, np.full(e_core - E_SHARD, rk[-1], dtype=rk.dtype)])
        ranks_pad.append(rp)
    nch2g = max(28, max(-(-len(r) // GROUP) for r in refs))
    LS2 = nch2g * GROUP
    NCH = -(-LS2 // CHUNK_STEP)
    chunk_meta = []
    for c in range(NCH):
        base = max(0, c * CHUNK_STEP - CHUNK_MARGIN)
        end = min(LS2, c * CHUNK_STEP - CHUNK_MARGIN + CHUNK_SPAN)
        chunk_meta.append((base, end - base))
    chunk_meta = tuple(chunk_meta)

    # shared (across cores) per-tile chunk choice
    chunk_of = []
    prev = 0
    for t in range(n_tiles):
        lo_t = min(int(rp[t * T]) for rp in ranks_pad)
        hi_t = max(int(rp[t * T + T - 1]) for rp in ranks_pad)
        pick = None
        for c in range(prev, NCH):
            base, rows = chunk_meta[c]
            if base <= lo_t and hi_t < base + rows:
                pick = c
                break
        assert pick is not None, (t, lo_t, hi_t, chunk_meta)
        chunk_of.append(pick)
        prev = pick
    chunk_of = tuple(chunk_of)

    # group emission schedule: each A2 group must be written before the first
    # tile that gathers from any chunk containing it
    first_tile = {}
    for t, c in enumerate(chunk_of):
        first_tile.setdefault(c, t)
    # chunks never picked: deadline of the next picked chunk (or end)
    d = [first_tile.get(c, n_tiles) for c in range(NCH)]
    for c in range(NCH - 2, -1, -1):
        d[c] = min(d[c], d[c + 1])
    tgt = {g: [c for c in range(NCH)
               if chunk_meta[c][0] <= g * GROUP
               and (g + 1) * GROUP <= chunk_meta[c][0] + chunk_meta[c][1]]
           for g in range(nch2g)}
    deadline = {g: min(d[c] for c in tgt[g]) for g in range(nch2g)}
    pre0 = tuple(sorted(g for g in range(nch2g) if deadline[g] <= 3))
    fill = tuple(sorted(g for g in range(nch2g) if 3 < deadline[g] <= 12))
    stage_at = [[] for _ in range(n_tiles)]
    compute_at = [[] for _ in range(n_tiles)]
    jit = sorted((g for g in range(nch2g) if deadline[g] > 12),
                 key=lambda g: (deadline[g], g))
    by_dl = {}
    for g in jit:
        by_dl.setdefault(deadline[g], []).append(g)
    for dl, gs in by_dl.items():
        # 2 groups per slot, last slot = deadline - SCHED_CUSHION
        slot = dl - SCHED_CUSHION
        for i, g in enumerate(reversed(gs)):
            s = max(1, min(slot - i // 2, n_tiles - 1))
            compute_at[s].append(g)
            stage_at[s - 1].append(g)
    pre0 = tuple(pre0)
    fill = tuple(fill)
    stage_at = tuple(tuple(s) for s in stage_at)
    compute_at = tuple(tuple(s) for s in compute_at)

    shared = _prep_shared(W1, b1, W2, b2, W3, b3)
    in_maps = []
    for c in range(N_CORES):
        lo = los[c]
        sl = slice(starts[c], starts[c] + E_SHARD)
        stl = np.zeros((SITE_LEN, LSITE), dtype=BF)
        avail = min(LSITE, N_SITES - lo)
        stl[:, :avail] = sitesT_bf[:, lo:lo + avail]

        st2 = np.zeros((SITE_LEN, LS2), dtype=BF)
        st2[:, :len(refs[c])] = sitesT_bf[:, refs[c]]

        i1_loc = np.zeros(e_core, dtype=np.int64)
        i1_loc[:E_SHARD] = i1s[sl] - lo
        # idx2: rank adjusted to be chunk-relative per tile
        i2_adj = ranks_pad[c].astype(np.int64, copy=True)
        for t in range(n_tiles):
            base = chunk_meta[chunk_of[t]][0]
            blk = slice(t * T, (t + 1) * T)
            i2_adj[blk] -= base
            assert i2_adj[blk].min() >= 0
            assert i2_adj[blk].max() < chunk_meta[chunk_of[t]][1]
        gb_pad = np.zeros(e_core, dtype=np.int64)
        gb_pad[:E_SHARD] = gbs[sl]
        bT = np.zeros((BOND_LEN, e_core), dtype=BF)
        bT[:, :E_SHARD] = bondsT_bf[:, sl]

        m = {
            "sitesT2": st2,
            "sitesTloc": stl,
            "statesT": statesT_bf,
            "bondsT": bT,
            "idx1": _wrap_idx(i1_loc),
            "idx2": _wrap_idx(i2_adj),
            "g2b": _wrap_idx(gb_pad),
        }
        m.update(shared)
        in_maps.append(m)

    key = (n_tiles, nloc, nch2g, chunk_meta, chunk_of, pre0, fill,
           stage_at, compute_at)
    nc = _get_bass(key, n_tiles, nloc, nch2g, chunk_meta, chunk_of,
                   pre0, fill, stage_at, compute_at)
    return nc, in_maps, perm, n_tiles


def kernel(sites, bonds, states, indices1, indices2, graph_to_bonds,
           W1, b1, W2, b2, W3, b3):
    nc, in_maps, perm, n_tiles = prepare(
        sites, bonds, states, indices1, indices2, graph_to_bonds,
        W1, b1, W2, b2, W3, b3)
    res = run_bass_kernel_spmd(nc, in_maps, core_ids=list(range(N_CORES)))
    out = np.empty((N_BONDS, OUT_DIM), dtype=np.float32)
    for c in range(N_CORES):
        sl = slice(c * E_SHARD, (c + 1) * E_SHARD)
        out[perm[sl]] = res.results[c]["outT"][:, :E_SHARD].T
    return out


# revision 20
# speedup vs baseline: 1.0355x; 1.0022x over previous
"""Trainium2 Bass kernel for BondUpdate GNN message passing.

Computes, for each bond e:
    x = concat(sites[idx1[e]], sites[idx2[e]], bonds[e], states[g2b[e]])  # [896]
    out[e] = relu(relu(relu(x @ W1 + b1) @ W2 + b2) @ W3 + b3)           # [256]

Strategy (v3): the 20000 sites are referenced ~400k times via idx1/idx2, and
512 graph states ~200k times, so the W1 blocks that multiply site/state
features are precomputed per core into DRAM projection tables:
    A1 = sites @ W1[0:256]      (only the local idx1 range)
    A2 = sites @ W1[256:512]    (deduped idx2 sites for this core)
    Ast = states @ W1[768:896]
Per bond, layer 1 then reduces to a 256-wide matmul on the bond features plus
a gather+sum of three projection rows, transposed into feature-major via the
PE. Bonds are sharded across 8 cores by idx1 range (so A1 is small), and
WITHIN each core sorted by idx2, so each tile's A2 rows form a narrow window
of the (rank-ordered) dedup table. The A2 table is then built as overlapping
2048-rank chunks (span 3584 rows) in separate DRAM tensors, and chunk
projection work is interleaved INTO the main loop just-in-time: the tensor
engine never sits idle waiting for the whole A2 table (the v2 kernel lost
~75us to precompute-phase stalls). Chunk choice per tile is shared across
cores (SPMD single program): the +-512-rank margin absorbs cross-core rank
fluctuations.

Activations stay transposed in SBUF (features on partitions, bonds on free
dim) so the three matmul layers chain without intermediate transposes.
Matmul operands are bf16 (PSUM accumulation fp32, biases+relu applied in
fp32, final output stage f32r so values are not re-rounded).
"""
import sys

if "/opt/trn_rl_repo" not in sys.path:
    sys.path.insert(0, "/opt/trn_rl_repo")

import ml_dtypes
import numpy as np

import concourse.bass as bass
import concourse.mybir as mybir
import concourse.tile as tile
from concourse.bass_utils import run_bass_kernel_spmd
from concourse.masks import make_identity
from concourse.vector_clock import ScopedClock

F32 = mybir.dt.float32
F32R = mybir.dt.float32r
BF16 = mybir.dt.bfloat16
I32 = mybir.dt.int32

P = 128            # partitions
T = 512            # bonds per tile
SUB = T // P       # 128-bond subtiles per tile

N_SITES = 20000
N_GRAPHS = 512
SITE_LEN = 256
BOND_LEN = 256
STATE_LEN = 128
H1 = 1024
H2 = 1024
OUT_DIM = 256

KCB, MC1 = BOND_LEN // P, H1 // P  # 2, 8   (bond block of W1)
KC2, MC2 = H1 // P, H2 // P        # 8, 8
KC3, MC3 = H2 // P, OUT_DIM // P   # 8, 2

N_CORES = 8
N_BONDS = 200000
E_SHARD = N_BONDS // N_CORES       # 25000 bonds per core
TILES_PER_CORE = 49                # 49*512 = 25088 >= 25000
GROUP = 512                        # table rows per projection group
CHUNK_STEP = 2048                  # rank stride between A2 chunks
CHUNK_MARGIN = 512                 # low-side rank margin per chunk
CHUNK_SPAN = 3584                  # rows covered by one chunk tensor
SCHED_CUSHION = 3                  # emit chunk groups >= this many tiles early

EVSEM_WAIT_CAP = 2  # InstEventSemaphore holds 2 waits; every other inst 1


def _legalize_waits(nc: bass.Bass):
    """Spill sync waits beyond the per-instruction capacity onto standalone
    InstEventSemaphore instructions inserted just before the offender.
    Walrus here rejects instructions with more waits than the ISA slots."""
    n_spilled = 0
    for f in nc.m.functions:
        for bb in f.blocks:
            il = bb.instructions
            i = 0
            while i < len(il):
                inst = il[i]
                si = inst.sync_info
                waits = list(si.on_wait) if si and si.on_wait else []
                cap = (
                    EVSEM_WAIT_CAP
                    if isinstance(inst, mybir.InstEventSemaphore)
                    else 1
                )
                if len(waits) > cap:
                    keep = waits[-cap:]
                    spill = waits[:-cap]
                    si.on_wait = keep
                    evs = []
                    for j in range(0, len(spill), EVSEM_WAIT_CAP):
                        ev = mybir.InstEventSemaphore(
                            name=nc.get_next_instruction_name(),
                            ins=[],
                            outs=[],
                            sync_info=mybir.SyncInfo(
                                on_wait=spill[j:j + EVSEM_WAIT_CAP],
                                on_update=[],
                            ),
                        )
                        ev.engine = inst.engine
                        nc.register_instruction(ev)
                        evs.append(ev)
                    il[i:i] = evs
                    i += len(evs)
                    n_spilled += len(spill)
                i += 1
    return n_spilled


class SplitDrainTileContext(tile.TileContext):
    """TileContext whose kernel-tail drain also respects the wait cap."""

    def _drain_and_barrier(self, tick_clock, wait_clock):
        nc = self.nc
        drain_inst = nc.sync.drain()
        wait_clock.add_sem_waits(
            drain_inst.ins, ScopedClock({None: tick_clock.global_clock})
        )
        si = drain_inst.ins.sync_info
        waits = list(si.on_wait or [])
        if len(waits) > 1:
            si.on_wait = []
            id2sem = {s.num: s for s in self.sems.allocated().values()}
            for w in waits:
                assert w.wait_mode == "sem-ge-imm", w
                nc.sync.wait_ge(id2sem[w.id], w.wait_value)
        nc.all_engine_barrier()
        assert self.sems is not None
        popped = nc._tile_sem_poison_stack.pop()
        assert popped is self._sem_poison
        nc.clear_and_free_semaphores(list(self.sems.allocated().values()))
        nc.all_engine_barrier()


def build_bass(n_tiles: int, nloc: int, nch2g: int,
               chunk_meta: tuple, chunk_of: tuple,
               pre0: tuple, fill: tuple,
               stage_at: tuple, compute_at: tuple) -> bass.Bass:
    """Per-core Bass program.

    chunk_meta: tuple of (base_row, n_rows) per A2 chunk tensor.
    chunk_of:   per tile, which chunk its idx2 gathers read.
    pre0:       A2 group ids computed before the gather prefetches (chunk 0).
    fill:       A2 group ids computed pre-main as latency filler.
    stage_at:   per tile, A2 group ids whose staging DMAs issue before
                that tile's body.
    compute_at: per tile, A2 group ids whose matmuls+writes are emitted
                after that tile's body.
    """
    nc = bass.Bass("TRN2", target_bir_lowering=False, debug=False, num_devices=1)
    E = n_tiles * T
    LSITE = nloc * P
    LS2 = nch2g * GROUP
    NCH = len(chunk_meta)

    # --- external inputs
    sitesT2 = nc.dram_tensor("sitesT2", [SITE_LEN, LS2], BF16, kind="ExternalInput")
    sitesTloc = nc.dram_tensor("sitesTloc", [SITE_LEN, LSITE], BF16, kind="ExternalInput")
    statesT = nc.dram_tensor("statesT", [STATE_LEN, N_GRAPHS], BF16, kind="ExternalInput")
    bondsT = nc.dram_tensor("bondsT", [BOND_LEN, E], BF16, kind="ExternalInput")
    # indices pre-wrapped on host to [P, n_tiles*SUB]: idx[p, t*SUB+j] = raw[t*T + j*P + p]
    idx1 = nc.dram_tensor("idx1", [P, n_tiles * SUB], I32, kind="ExternalInput")
    idx2 = nc.dram_tensor("idx2", [P, n_tiles * SUB], I32, kind="ExternalInput")
    g2b = nc.dram_tensor("g2b", [P, n_tiles * SUB], I32, kind="ExternalInput")
    # W1 site block rows 0:512 as [p, k, h] = W1[k*128+p, h], k=0..3 (bf16)
    w1s = nc.dram_tensor("w1s", [P, 4, H1], BF16, kind="ExternalInput")
    # W1 state block rows 768:896: [p, h] = W1[768+p, h]
    w1st = nc.dram_tensor("w1st", [P, H1], BF16, kind="ExternalInput")
    # W1 bond block rows 512:768 chunked: w1c[p, (k*MC1+m)*P+j] = W1[512+k*P+p, m*P+j]
    w1c = nc.dram_tensor("w1c", [P, KCB * MC1 * P], BF16, kind="ExternalInput")
    w2c = nc.dram_tensor("w2c", [P, KC2 * MC2 * P], BF16, kind="ExternalInput")
    w3c = nc.dram_tensor("w3c", [P, KC3 * MC3 * P], BF16, kind="ExternalInput")
    # biases pre-wrapped: bXc[p, m] = bX[m*P+p]
    b1c = nc.dram_tensor("b1c", [P, MC1], F32, kind="ExternalInput")
    b2c = nc.dram_tensor("b2c", [P, MC2], F32, kind="ExternalInput")
    b3c = nc.dram_tensor("b3c", [P, MC3], F32, kind="ExternalInput")
    outT = nc.dram_tensor("outT", [OUT_DIM, E], F32, kind="ExternalOutput")

    # --- internal DRAM projection tables (bf16 rows, gathered per bond)
    A1d = nc.dram_tensor("A1d", [LSITE, H1], BF16, kind="Internal")
    Astd = nc.dram_tensor("Astd", [N_GRAPHS, H1], BF16, kind="Internal")
    A2ch = []
    for c, (_base, rows) in enumerate(chunk_meta):
        A2ch.append(nc.dram_tensor(f"A2d{c}", [rows, H1], BF16, kind="Internal"))

    with SplitDrainTileContext(nc) as tc:
        with (
            tc.tile_pool(name="const", bufs=1) as constp,
            tc.tile_pool(name="wts", bufs=1) as wp,
            tc.tile_pool(name="idx", bufs=1) as idxp,
            tc.tile_pool(name="pstage", bufs=6) as pstage,
            tc.tile_pool(name="aout", bufs=10) as aoutp,
            tc.tile_pool(name="gath1", bufs=4) as g1p,
            tc.tile_pool(name="gath2", bufs=2) as g2p,
            tc.tile_pool(name="ssum", bufs=2) as ssump,
            tc.tile_pool(name="s01p", bufs=1) as s01p,
            tc.tile_pool(name="xT", bufs=3) as xp,
            tc.tile_pool(name="sT", bufs=2) as stp,
            tc.tile_pool(name="acts", bufs=1) as hp,
            tc.tile_pool(name="psmm", bufs=6, space="PSUM") as psmm,
            tc.tile_pool(name="psx", bufs=2, space="PSUM") as psx,
        ):
            # ---- startup loads: w1s k2/k3 first (A2 chunk-0 groups run
            # first), idx on the scalar queue so sync is free for staging
            ident_bf = constp.tile([P, P], BF16)
            make_identity(nc, ident_bf[:])

            w1s_sb = wp.tile([P, 4, H1], BF16)
            for k in (2, 3, 0, 1):
                nc.gpsimd.dma_start(w1s_sb[:, k, :], w1s[:, k, :])
            w1st_sb = wp.tile([P, H1], BF16)
            nc.gpsimd.dma_start(w1st_sb[:], w1st[:, :])

            b1sb = constp.tile([P, MC1], F32)
            b2sb = constp.tile([P, MC2], F32)
            b3sb = constp.tile([P, MC3], F32)
            nc.scalar.dma_start(b1sb[:], b1c[:, :])
            nc.scalar.dma_start(b2sb[:], b2c[:, :])
            nc.scalar.dma_start(b3sb[:], b3c[:, :])

            idx1sb = idxp.tile([P, n_tiles * SUB], I32)
            idx2sb = idxp.tile([P, n_tiles * SUB], I32)
            g2bsb = idxp.tile([P, n_tiles * SUB], I32)
            nc.scalar.dma_start(idx2sb[:], idx2[:, :])
            nc.scalar.dma_start(idx1sb[:], idx1[:, :])
            nc.scalar.dma_start(g2bsb[:], g2b[:, :])

            w1sb = wp.tile([P, KCB * MC1 * P], BF16)
            w2sb = wp.tile([P, KC2 * MC2 * P], BF16)
            w3sb = wp.tile([P, KC3 * MC3 * P], BF16)

            def stage_unit(src_dram, src_col0, nst):
                sts = []
                for s in range(nst):
                    st = pstage.tile([P, 4 * P], BF16, tag=f"st{s}")
                    nc.sync.dma_start(
                        st[:], src_dram[s * P:(s + 1) * P,
                                        src_col0:src_col0 + 4 * P])
                    sts.append(st)
                return sts

            def compute_unit(sts, ks, dsts):
                """Project 512 staged table rows through w1s chunk(s) ks,
                write bf16 rows to every (dram, row_offset) in dsts. PSUM
                drains via the vector engine (idle at group-emission points)."""
                for i in range(4):
                    ao = aoutp.tile([P, H1], BF16, tag="ao")
                    for h in range(2):
                        ps = psmm.tile([P, T], F32, tag="psmm")
                        hs = slice(h * 512, (h + 1) * 512)
                        for si, (st, k) in enumerate(zip(sts, ks)):
                            nc.tensor.matmul(
                                ps[:], st[:, i * P:(i + 1) * P],
                                w1s_sb[:, k, hs] if k >= 0 else w1st_sb[:, hs],
                                start=(si == 0), stop=(si == len(sts) - 1),
                            )
                        nc.vector.tensor_copy(ao[:, hs], ps[:])
                    for (dst, roff) in dsts:
                        nc.sync.dma_start(dst[roff + i * P: roff + (i + 1) * P, :], ao[:])

            def a2_dsts(g):
                dsts = []
                for c, (base, rows) in enumerate(chunk_meta):
                    off = g * GROUP - base
                    if 0 <= off and off + GROUP <= rows:
                        dsts.append((A2ch[c], off))
                assert dsts, f"A2 group {g} maps to no chunk"
                return dsts

            def stage_a2(g):
                return stage_unit(sitesT2, g * GROUP, 2)

            def compute_a2(g, sts):
                compute_unit(sts, (2, 3), a2_dsts(g))

            staged = {}

            PREF = 2

            def issue_g1(t, a1g, asg):
                for j in range(SUB):
                    cj = t * SUB + j
                    nc.gpsimd.indirect_dma_start(
                        out=a1g[:, j, :], out_offset=None, in_=A1d[:],
                        in_offset=bass.IndirectOffsetOnAxis(
                            ap=idx1sb[:, cj:cj + 1], axis=0),
                    )
                    nc.gpsimd.indirect_dma_start(
                        out=asg[:, j, :], out_offset=None, in_=Astd[:],
                        in_offset=bass.IndirectOffsetOnAxis(
                            ap=g2bsb[:, cj:cj + 1], axis=0),
                    )

            def issue_g2(t, a2g):
                src = A2ch[chunk_of[t]]
                for j in range(SUB):
                    cj = t * SUB + j
                    nc.gpsimd.indirect_dma_start(
                        out=a2g[:, j, :], out_offset=None, in_=src[:],
                        in_offset=bass.IndirectOffsetOnAxis(
                            ap=idx2sb[:, cj:cj + 1], axis=0),
                    )

            # ---- pre-main pipeline: chunk-0 groups, A1, Ast, then chunk-1
            # filler groups that keep the PE busy while the first tiles'
            # gathers+adds land. Staging runs 4 units ahead on sync.
            units = ([("a2", g) for g in pre0]
                     + [("a1", g) for g in range(nloc // 4)]
                     + [("ast", 0)]
                     + [("a2", g) for g in fill])

            def stage_pre(u):
                kind, g = u
                if kind == "a2":
                    staged[g] = stage_a2(g)
                elif kind == "a1":
                    staged[("a1", g)] = stage_unit(sitesTloc, g * GROUP, 2)
                else:
                    staged[("ast", 0)] = stage_unit(statesT, 0, 1)

            pre_g = {}
            pre_a2 = {}
            LOOKAHEAD = 4
            for u in units[:LOOKAHEAD]:
                stage_pre(u)
            n_pre0 = len(pre0)
            for ui, u in enumerate(units):
                kind, g = u
                if kind == "a2":
                    compute_a2(g, staged.pop(g))
                elif kind == "a1":
                    compute_unit(staged.pop(("a1", g)), (0, 1),
                                 [(A1d, g * GROUP)])
                else:
                    compute_unit(staged.pop(("ast", 0)), (-1,), [(Astd, 0)])
                if ui + LOOKAHEAD < len(units):
                    stage_pre(units[ui + LOOKAHEAD])
                if ui == n_pre0 - 1:
                    # mini chunk written: prefetch the idx2 gathers of the
                    # tiles it serves (waiting on any later chunk here would
                    # head-of-line-block the gpsimd queue)
                    for t in range(min(2, n_tiles)):
                        if chunk_of[t] != 0:
                            break
                        a2g = g2p.tile([P, SUB, H1], BF16, tag="a2g")
                        issue_g2(t, a2g)
                        pre_a2[t] = a2g
                    # weight loads for the main loop ride behind on gpsimd
                    nc.gpsimd.dma_start(w1sb[:], w1c[:, :])
                if ui == n_pre0 + nloc // 4:
                    # A1 + Ast written: prefetch the first tiles' row gathers
                    for t in range(min(PREF, n_tiles)):
                        a1g = g1p.tile([P, SUB, H1], BF16, tag="a1g")
                        asg = g1p.tile([P, SUB, H1], BF16, tag="asg")
                        issue_g1(t, a1g, asg)
                        pre_g[t] = (a1g, asg)
                    nc.scalar.dma_start(w2sb[:], w2c[:, :])
                    nc.scalar.dma_start(w3sb[:], w3c[:, :])

            # ================= main loop =================
            for t in range(n_tiles):
                # staging DMAs for groups computed after the NEXT tile
                for g in stage_at[t]:
                    staged[g] = stage_a2(g)
                # ---- gather projection rows: [P, SUB, H1] bf16
                if t in pre_g:
                    a1g, asg = pre_g.pop(t)
                else:
                    a1g = g1p.tile([P, SUB, H1], BF16, tag="a1g")
                    asg = g1p.tile([P, SUB, H1], BF16, tag="asg")
                    issue_g1(t, a1g, asg)
                if t in pre_a2:
                    a2g = pre_a2.pop(t)
                else:
                    a2g = g2p.tile([P, SUB, H1], BF16, tag="a2g")
                    issue_g2(t, a2g)
                # ---- sum the three projections (still bond-major)
                s01 = s01p.tile([P, SUB, H1], BF16, tag="s01")
                ssum = ssump.tile([P, SUB, H1], BF16, tag="ssum")
                nc.vector.tensor_add(s01[:], a1g[:], a2g[:])
                nc.vector.tensor_add(ssum[:], s01[:], asg[:])

                # bonds arrive pre-transposed from the host: cast-DMA chunks
                xb = []
                for c in range(KCB):
                    xsb = xp.tile([P, T], BF16, tag=f"xTb{c}")
                    nc.sync.dma_start(
                        xsb[:], bondsT[c * P:(c + 1) * P, t * T:(t + 1) * T])
                    xb.append(xsb)

                # ---- layer 1: bond-block matmul + transposed projection sum
                h1T = []
                for m in range(MC1):
                    ps = psmm.tile([P, T], F32, tag="psmm")
                    for k in range(KCB):
                        nc.tensor.matmul(
                            ps[:],
                            w1sb[:, (k * MC1 + m) * P:(k * MC1 + m + 1) * P],
                            xb[k][:],
                            start=(k == 0), stop=(k == KCB - 1),
                        )
                    pst = psx.tile([P, T], BF16, tag="psx")
                    for j in range(SUB):
                        nc.tensor.transpose(
                            pst[:, j * P:(j + 1) * P],
                            ssum[:, j, m * P:(m + 1) * P],
                            ident_bf[:],
                        )
                    sT = stp.tile([P, T], BF16, tag="sT")
                    nc.vector.tensor_copy(sT[:], pst[:])
                    pre = stp.tile([P, T], BF16, tag="pre")
                    nc.vector.tensor_add(pre[:], ps[:], sT[:])
                    hsb = hp.tile([P, T], BF16, tag=f"h1T{m}")
                    nc.scalar.activation(
                        hsb[:], pre[:], mybir.ActivationFunctionType.Relu,
                        bias=b1sb[:, m:m + 1],
                    )
                    h1T.append(hsb)

                # ---- layer 2
                h2T = []
                for m in range(MC2):
                    ps = psmm.tile([P, T], F32, tag="psmm")
                    for k in range(KC2):
                        nc.tensor.matmul(
                            ps[:],
                            w2sb[:, (k * MC2 + m) * P:(k * MC2 + m + 1) * P],
                            h1T[k][:],
                            start=(k == 0), stop=(k == KC2 - 1),
                        )
                    hsb = hp.tile([P, T], BF16, tag=f"h2T{m}")
                    nc.scalar.activation(
                        hsb[:], ps[:], mybir.ActivationFunctionType.Relu,
                        bias=b2sb[:, m:m + 1],
                    )
                    h2T.append(hsb)

                # ---- layer 3
                oT = []
                for m in range(MC3):
                    ps = psmm.tile([P, T], F32, tag="psmm")
                    for k in range(KC3):
                        nc.tensor.matmul(
                            ps[:],
                            w3sb[:, (k * MC3 + m) * P:(k * MC3 + m + 1) * P],
                            h2T[k][:],
                            start=(k == 0), stop=(k == KC3 - 1),
                        )
                    hsb = hp.tile([P, T], F32R, tag=f"oT{m}")
                    nc.scalar.activation(
                        hsb[:], ps[:], mybir.ActivationFunctionType.Relu,
                        bias=b3sb[:, m:m + 1],
                    )
                    oT.append(hsb)

                # ---- store transposed output; host un-transposes
                for c in range(MC3):
                    nc.sync.dma_start(
                        outT[c * P:(c + 1) * P, t * T:(t + 1) * T],
                        oT[c][:].bitcast(F32),
                    )

                # ---- just-in-time A2 chunk projections for upcoming tiles
                for g in compute_at[t]:
                    compute_a2(g, staged.pop(g))

    _legalize_waits(nc)
    return nc


def _prep_shared(W1, b1, W2, b2, W3, b3):
    BF = ml_dtypes.bfloat16
    W1 = np.asarray(W1, dtype=np.float32)

    def chunk_w(W, KC, MC):
        # [KC*P, MC*P] -> [P, KC*MC*P] with w[p, (k*MC+m)*P+j] = W[k*P+p, m*P+j]
        return np.ascontiguousarray(
            W.reshape(KC, P, MC, P).transpose(1, 0, 2, 3).reshape(P, KC * MC * P)
        ).astype(BF)

    def chunk_b(b, MC):
        return np.ascontiguousarray(np.asarray(b).reshape(MC, P).T).astype(
            np.float32, copy=False)

    return {
        "w1s": np.ascontiguousarray(
            W1[0:512].reshape(4, P, H1).transpose(1, 0, 2)).astype(BF),
        "w1st": np.ascontiguousarray(W1[768:896]).astype(BF),
        "w1c": chunk_w(W1[512:768], KCB, MC1),
        "w2c": chunk_w(np.asarray(W2, dtype=np.float32), KC2, MC2),
        "w3c": chunk_w(np.asarray(W3, dtype=np.float32), KC3, MC3),
        "b1c": chunk_b(b1, MC1),
        "b2c": chunk_b(b2, MC2),
        "b3c": chunk_b(b3, MC3),
    }


def _wrap_idx(raw: np.ndarray) -> np.ndarray:
    # [E_core] -> [P, n_tiles*SUB] with idx[p, q] = raw[q*P + p]
    n = raw.shape[0] // P
    return np.ascontiguousarray(raw.reshape(n, P).T).astype(np.int32, copy=False)


_BUILT = {}


def _get_bass(key, *args) -> bass.Bass:
    if key not in _BUILT:
        _BUILT[key] = build_bass(*args)
    return _BUILT[key]


def prepare(sites, bonds, states, indices1, indices2, graph_to_bonds,
            W1, b1, W2, b2, W3, b3):
    """Shard + reformat full inputs. Returns (nc, in_maps, perm, n_tiles)."""
    i1 = np.asarray(indices1).astype(np.int64, copy=False)
    i2 = np.asarray(indices2).astype(np.int64, copy=False)
    gb = np.asarray(graph_to_bonds).astype(np.int64, copy=False)
    bonds = np.asarray(bonds, dtype=np.float32)
    n_bonds = bonds.shape[0]
    assert n_bonds == N_BONDS

    # shard by idx1 range, then sort each shard by idx2 so A2 gathers sweep
    # the rank-ordered dedup table monotonically
    perm0 = np.argsort(i1, kind="stable")
    starts = [c * E_SHARD for c in range(N_CORES)]
    i1_sorted = i1[perm0]
    los = [int(i1_sorted[s]) for s in starts]
    his = [int(i1_sorted[s + E_SHARD - 1]) for s in starts]
    sizes = [hi - lo + 1 for lo, hi in zip(los, his)]
    nloc = max(20, 4 * (-(-max(sizes) // (4 * P))))
    LSITE = nloc * P

    perm = np.empty_like(perm0)
    for c in range(N_CORES):
        sl = slice(starts[c], starts[c] + E_SHARD)
        sub = perm0[sl]
        order = np.argsort(i2[sub], kind="stable")
        perm[sl] = sub[order]

    i1s, i2s, gbs = i1[perm], i2[perm], gb[perm]
    bondsT_s = np.ascontiguousarray(bonds[perm].T)  # [256, n_bonds]

    n_tiles = max(TILES_PER_CORE, -(-E_SHARD // T))
    e_core = n_tiles * T

    BF = ml_dtypes.bfloat16
    sitesT_bf = np.asarray(sites, dtype=np.float32).T.astype(BF)  # [256, N_SITES]
    statesT_bf = np.ascontiguousarray(
        np.asarray(states, dtype=np.float32).T).astype(BF)
    bondsT_bf = bondsT_s.astype(BF)

    # dedup idx2 per core; ranks are non-decreasing with steps in {0, 1}
    refs, ranks_pad = [], []
    for c in range(N_CORES):
        sl = slice(starts[c], starts[c] + E_SHARD)
        r = np.unique(i2s[sl])
        refs.append(r)
        rk = np.searchsorted(r, i2s[sl])
        rp = np.concatenate([rk, np.full(e_core - E_SHARD, rk[-1], dtype=rk.dtype)])
        ranks_pad.append(rp)
    nch2g = max(28, max(-(-len(r) // GROUP) for r in refs))
    LS2 = nch2g * GROUP
    NCH = -(-LS2 // CHUNK_STEP)
    # chunk 0 is a small 2-group tensor so the first tiles' gathers only
    # depend on ~2MB of table writes (the startup phase is DMA-bound)
    chunk_meta = [(0, 2 * GROUP)]
    for c in range(NCH):
        base = max(0, c * CHUNK_STEP - CHUNK_MARGIN)
        end = min(LS2, c * CHUNK_STEP - CHUNK_MARGIN + CHUNK_SPAN)
        chunk_meta.append((base, end - base))
    chunk_meta = tuple(chunk_meta)

    # shared (across cores) per-tile chunk choice
    NCHT = len(chunk_meta)
    chunk_of = []
    prev = 0
    for t in range(n_tiles):
        lo_t = min(int(rp[t * T]) for rp in ranks_pad)
        hi_t = max(int(rp[t * T + T - 1]) for rp in ranks_pad)
        pick = None
        for c in range(prev, NCHT):
            base, rows = chunk_meta[c]
            if base <= lo_t and hi_t < base + rows:
                pick = c
                break
        assert pick is not None, (t, lo_t, hi_t, chunk_meta)
        chunk_of.append(pick)
        prev = pick
    chunk_of = tuple(chunk_of)

    # group emission schedule: each A2 group must be written before the first
    # tile that gathers from any chunk containing it
    first_tile = {}
    for t, c in enumerate(chunk_of):
        first_tile.setdefault(c, t)
    # chunks never picked: deadline of the next picked chunk (or end)
    d = [first_tile.get(c, n_tiles) for c in range(NCHT)]
    for c in range(NCHT - 2, -1, -1):
        d[c] = min(d[c], d[c + 1])
    tgt = {g: [c for c in range(NCHT)
               if chunk_meta[c][0] <= g * GROUP
               and (g + 1) * GROUP <= chunk_meta[c][0] + chunk_meta[c][1]]
           for g in range(nch2g)}
    deadline = {g: min(d[c] for c in tgt[g]) for g in range(nch2g)}
    pre0 = tuple(sorted(g for g in range(nch2g) if deadline[g] <= 0))
    fill = tuple(sorted(g for g in range(nch2g) if 0 < deadline[g] <= 12))
    stage_at = [[] for _ in range(n_tiles)]
    compute_at = [[] for _ in range(n_tiles)]
    jit = sorted((g for g in range(nch2g) if deadline[g] > 12),
                 key=lambda g: (deadline[g], g))
    by_dl = {}
    for g in jit:
        by_dl.setdefault(deadline[g], []).append(g)
    for dl, gs in by_dl.items():
        # 2 groups per slot, last slot = deadline - SCHED_CUSHION
        slot = dl - SCHED_CUSHION
        for i, g in enumerate(reversed(gs)):
            s = max(1, min(slot - i // 2, n_tiles - 1))
            compute_at[s].append(g)
            stage_at[s - 1].append(g)
    pre0 = tuple(pre0)
    fill = tuple(fill)
    stage_at = tuple(tuple(s) for s in stage_at)
    compute_at = tuple(tuple(s) for s in compute_at)

    shared = _prep_shared(W1, b1, W2, b2, W3, b3)
    in_maps = []
    for c in range(N_CORES):
        lo = los[c]
        sl = slice(starts[c], starts[c] + E_SHARD)
        stl = np.zeros((SITE_LEN, LSITE), dtype=BF)
        avail = min(LSITE, N_SITES - lo)
        stl[:, :avail] = sitesT_bf[:, lo:lo + avail]

        st2 = np.zeros((SITE_LEN, LS2), dtype=BF)
        st2[:, :len(refs[c])] = sitesT_bf[:, refs[c]]

        i1_loc = np.zeros(e_core, dtype=np.int64)
        i1_loc[:E_SHARD] = i1s[sl] - lo
        # idx2: rank adjusted to be chunk-relative per tile
        i2_adj = ranks_pad[c].astype(np.int64, copy=True)
        for t in range(n_tiles):
            base = chunk_meta[chunk_of[t]][0]
            blk = slice(t * T, (t + 1) * T)
            i2_adj[blk] -= base
            assert i2_adj[blk].min() >= 0
            assert i2_adj[blk].max() < chunk_meta[chunk_of[t]][1]
        gb_pad = np.zeros(e_core, dtype=np.int64)
        gb_pad[:E_SHARD] = gbs[sl]
        bT = np.zeros((BOND_LEN, e_core), dtype=BF)
        bT[:, :E_SHARD] = bondsT_bf[:, sl]

        m = {
            "sitesT2": st2,
            "sitesTloc": stl,
            "statesT": statesT_bf,
            "bondsT": bT,
            "idx1": _wrap_idx(i1_loc),
            "idx2": _wrap_idx(i2_adj),
            "g2b": _wrap_idx(gb_pad),
        }
        m.update(shared)
        in_maps.append(m)

    key = (n_tiles, nloc, nch2g, chunk_meta, chunk_of, pre0, fill,
           stage_at, compute_at)
    nc = _get_bass(key, n_tiles, nloc, nch2g, chunk_meta, chunk_of,
                   pre0, fill, stage_at, compute_at)
    return nc, in_maps, perm, n_tiles


def kernel(sites, bonds, states, indices1, indices2, graph_to_bonds,
           W1, b1, W2, b2, W3, b3):
    nc, in_maps, perm, n_tiles = prepare(
        sites, bonds, states, indices1, indices2, graph_to_bonds,
        W1, b1, W2, b2, W3, b3)
    res = run_bass_kernel_spmd(nc, in_maps, core_ids=list(range(N_CORES)))
    out = np.empty((N_BONDS, OUT_DIM), dtype=np.float32)
    for c in range(N_CORES):
        sl = slice(c * E_SHARD, (c + 1) * E_SHARD)
        out[perm[sl]] = res.results[c]["outT"][:, :E_SHARD].T
    return out
